# revision 1
# baseline (speedup 1.0000x reference)
"""BatchHardTripletLoss on 8 Trainium2 NeuronCores.

Strategy (data-parallel over anchor rows, samples pre-sorted by label):
  - host sorts samples by label (loss is permutation-invariant); core c owns
    anchor rows [c*512, (c+1)*512).
  - sorting clusters each core's 512 rows into <=126 distinct label classes,
    so the same-label penalty matrix PEN * 1[label_m == label_n] restricted to
    a core's rows is RANK <= 126: PEN * onehot_m . onehot_n over class dims.
    It is injected by ONE extra K=128 matmul per output tile (values 128*128 =
    PEN exactly in fp16). Two spare class dims carry the column terms
    (|e_n|^2 - 2*eps*sum(e_n), split into fp16 hi+lo rows against ones).
  - per (m,n) tile: pen/colterm matmul (start=True) + 4 gram matmuls
    accumulate  w = colterm[n] + PEN*same - 2 e_m.e_n  directly in PSUM.
  - DVE mines row-max (hardest positive + PEN) and row-min (hardest negative)
    straight out of PSUM; row-constant terms are applied on the host.
  - host: subtract PEN, add row terms, sqrt, validity via label bincount, mean.
"""

import dataclasses

import numpy as np

import concourse.bacc as bacc
import concourse.mybir as mybir
from concourse.bass_utils import run_bass_kernel_spmd
from concourse.tile import TileContext
from concourse import dve_ops as _dve_ops
from concourse.dve_spec import (
    AluOp, C0, C1, C2, Idx, Spec, Src0, lower, ne, scan, select,
)
from concourse.dve_uop import DveOpSpec


def _register_dual_op():
    """One DVE pass over a [P, N] tile producing BOTH reductions:
    accum_out = max(seed_s1, body) where body = w except at the last index,
    out[:, N-1] = running min (seeded +FLT_MAX via imm2) = total row min.
    The true max of the last column is restored by a 1-element fixup op.
    """
    name = "ANT_MINMAX_DUAL"
    for op in _dve_ops.OPS:
        if op.name == name:
            return op
    spec = Spec(
        body=select(ne(Idx, C0), Src0, scan(AluOp.MIN, Src0, init=C2)),
        accum=AluOp.MAX,
        accum_init=C1,
        reference=lambda in0, s0, s1, imm2: np.where(
            np.arange(in0.shape[-1]) != s0,
            in0,
            np.minimum.accumulate(np.minimum(in0, imm2), axis=-1),
        ),
    )
    op = _dve_ops.DveOp(name, spec, subdim=False, uops_sha={})
    _dve_ops.OPS.append(op)
    opcode = _dve_ops._CUSTOM_DVE_ROW_BASE + len(_dve_ops.OPS) - 1
    assert opcode < 0x20
    _dve_ops._SUB_OPCODE_FOR_NAME[name] = opcode
    _dve_ops.CUSTOM_DVE_SPECS[name] = spec
    shas = {}
    for ver in ("v3", "v4"):
        s = DveOpSpec(name=name, opcode=opcode, uops=lower(spec, ver=ver),
                      rd1_en=False)
        shas[ver] = s.sha(ver)
    op = dataclasses.replace(op, uops_sha=shas)
    _dve_ops.OPS[-1] = op
    return op


DUAL_OP = _register_dual_op()

B = 4096          # batch (anchors)
D = 512           # embedding dim
N_CORES = 8
ROWS = B // N_CORES      # 512 anchor rows per core
P = 128                  # partitions
MT = ROWS // P           # 4 m-tiles per core
NW = 512                 # psum bank width (fp32)
GW = 2048                # column group width (4 banks)
NG = B // GW             # 2 column groups
KT = D // P              # 4 contraction tiles

PEN = 16384.0            # same-label penalty; must exceed max d2 (~2.7k here)
KP = 96                  # penalty-matmul contraction (class dims + 2)
NCLASS_MAX = KP - 2      # class dims per core (2 reserved for colterm hi/lo)
MARGIN = 0.5
EPS = 1e-6

_nc_cache = {}


def _build(reps=1, kp=KP):
    nc = bacc.Bacc("TRN2", target_bir_lowering=False)
    fp16 = mybir.dt.float16
    f32 = mybir.dt.float32

    et = nc.dram_tensor("et", [D, B], fp16, kind="ExternalInput")
    eblk = nc.dram_tensor("eblk", [D, ROWS], fp16, kind="ExternalInput")
    penl = nc.dram_tensor("penl", [kp, ROWS], fp16, kind="ExternalInput")
    penr = nc.dram_tensor("penr", [kp, B], fp16, kind="ExternalInput")
    outd = nc.dram_tensor("out", [reps, 2 * MT, P], f32, kind="ExternalOutput")

    NEG_INIT = -3.0e38
    SCAN_INIT = 3.0e38
    with TileContext(nc) as tc:
        with (
            tc.tile_pool(name="etp", bufs=1) as etp,
            tc.tile_pool(name="ebp", bufs=1) as ebp,
            tc.tile_pool(name="wp", bufs=2) as wp,
            tc.tile_pool(name="accp", bufs=MT) as accp,
            tc.tile_pool(name="psp", bufs=2, space="PSUM") as psp,
        ):
            # --- PE warmup: dense tiny matmuls while input DMAs run -------
            warm = etp.tile([P, 64], fp16, tag="warm")
            nc.vector.memset(warm, 0.0)
            wps = psp.tile([P, GW], f32, tag="ps", name="wps")
            for _ in range(40):
                nc.tensor.matmul(wps[:64, 0:64], warm[:, 0:64], warm[:, 0:64],
                                 start=True, stop=True)

            # --- input DMAs, few large transfers, critical-path first -----
            penl_sb = etp.tile([kp, ROWS], fp16, tag="penl")
            nc.gpsimd.dma_start(out=penl_sb, in_=penl[:, :])
            penr_sb = etp.tile([kp, B], fp16, tag="penr")
            nc.gpsimd.dma_start(out=penr_sb[:, 0:GW], in_=penr[:, 0:GW])
            eb_all = ebp.tile([P, KT * ROWS], fp16, tag="eb", name="eb_all")
            et_all = etp.tile([P, KT * B], fp16, tag="et", name="et_all")
            eb_sb = [eb_all[:, k * ROWS:(k + 1) * ROWS] for k in range(KT)]
            et_sb = [et_all[:, k * B:(k + 1) * B] for k in range(KT)]
            eb4 = eb_all.rearrange("p (k n) -> p k n", k=KT)
            et4 = et_all.rearrange("p (k n) -> p k n", k=KT)
            ebd4 = eblk.rearrange("(k p) n -> p k n", p=P)
            etd4 = et.rearrange("(k p) n -> p k n", p=P)
            nc.gpsimd.dma_start(out=eb4, in_=ebd4)
            nc.gpsimd.dma_start(out=et4[:, :, 0:GW], in_=etd4[:, :, 0:GW])
            nc.gpsimd.dma_start(out=penr_sb[:, GW:B], in_=penr[:, GW:B])
            nc.gpsimd.dma_start(out=et4[:, :, GW:B], in_=etd4[:, :, GW:B])
            ident = etp.tile([P, P], f32, tag="ident")
            from concourse.masks import make_identity
            make_identity(nc, ident)

            for r in range(reps):
                out_sb = accp.tile([P, 2 * MT], f32, tag="osb", name="osb")
                hp_accs = [accp.tile([P, NG], f32, tag="hp", name=f"hp{t}")
                           for t in range(MT)]
                hn_accs = [accp.tile([P, NG], f32, tag="hn", name=f"hn{t}")
                           for t in range(MT)]
                for g in range(NG):
                  for t in range(MT):
                    ms = slice(t * P, (t + 1) * P)
                    hp_acc, hn_acc = hp_accs[t], hn_accs[t]
                    last = g == NG - 1
                    ps = psp.tile([P, GW], f32, tag="ps", name="ps")
                    # penalty + column-term injection (start=True clears bank)
                    for j in range(GW // NW):
                        cs = slice(g * GW + j * NW, g * GW + (j + 1) * NW)
                        nc.tensor.matmul(
                            ps[:, j * NW:(j + 1) * NW],
                            penl_sb[:, ms], penr_sb[:, cs],
                            start=True, stop=False,
                        )
                    # gram accumulation: w = colterm + PEN*same - 2 e_m.e_n
                    for k in range(KT):
                        for j in range(GW // NW):
                            cs = slice(g * GW + j * NW, g * GW + (j + 1) * NW)
                            nc.tensor.matmul(
                                ps[:, j * NW:(j + 1) * NW],
                                eb_sb[k][:, ms], et_sb[k][:, cs],
                                start=False, stop=(k == KT - 1),
                            )
                    # fused mining: accum_out = row-max (chained via s1),
                    # scratch[:, -1] = row-min (scan), fixup col GW-1 into max
                    scratch = wp.tile([P, GW], f32, tag="scr", name="scr")
                    nc.vector._custom_dve(
                        DUAL_OP,
                        out=scratch,
                        in0=ps,
                        s0=float(GW - 1),
                        s1=(NEG_INIT if g == 0 else hp_acc[:, g - 1:g]),
                        imm2=SCAN_INIT,
                        accum_out=hp_acc[:, g:g + 1],
                    )
                    # restore the excluded last column into the max
                    nc.vector.tensor_tensor(
                        out_sb[:, t:t + 1] if last else hp_acc[:, g:g + 1],
                        ps[:, GW - 1:GW],
                        hp_acc[:, g:g + 1], mybir.AluOpType.max,
                    )
                    # chain the min across groups
                    if g == 0:
                        nc.vector.tensor_copy(
                            hn_acc[:, 0:1], scratch[:, GW - 1:GW])
                    else:
                        nc.vector.tensor_tensor(
                            out_sb[:, MT + t:MT + t + 1] if last
                            else hn_acc[:, g:g + 1],
                            scratch[:, GW - 1:GW],
                            hn_acc[:, g - 1:g], mybir.AluOpType.min,
                        )
                # pack outputs: [128, 8] -> [8, 128] via PE transpose, one DMA
                tr = psp.tile([P, GW], f32, tag="ps", name="tr")
                nc.tensor.transpose(tr[0:2 * MT, 0:P], out_sb, ident)
                out_tr = accp.tile([P, P], f32, tag="otr", name="otr")
                nc.vector.tensor_copy(out_tr[0:2 * MT, :], tr[0:2 * MT, 0:P])
                nc.sync.dma_start(out=outd[r], in_=out_tr[0:2 * MT, :])
    nc.compile()
    return nc


def _get_nc(reps=1, kp=KP):
    if (reps, kp) not in _nc_cache:
        _nc_cache[(reps, kp)] = _build(reps, kp)
    return _nc_cache[(reps, kp)]


def _prepare_inputs(embeddings, labels):
    Ef = np.ascontiguousarray(np.asarray(embeddings, dtype=np.float32))
    lab = np.asarray(labels).astype(np.int64)
    perm = np.argsort(lab, kind="stable")
    Ef = Ef[perm]
    labp = lab[perm]

    sq = np.sum(Ef * Ef, axis=1, dtype=np.float32)          # [B]
    s = np.sum(Ef, axis=1, dtype=np.float32)                # [B]
    et16 = np.ascontiguousarray(
        (Ef * np.float32(np.sqrt(2.0))).T.astype(np.float16))   # [D, B]


    colterm = (sq - 2.0 * EPS * s).astype(np.float32)
    colhi = colterm.astype(np.float16)
    collo = (colterm - colhi.astype(np.float32)).astype(np.float16)
    rowterm = (sq + 2.0 * EPS * s + D * EPS * EPS).astype(np.float32)

    # global class segments in sorted order
    # seg_start[q], seg_end[q] for each distinct label value
    uniq, first = np.unique(labp, return_index=True)
    bounds = np.r_[first, B]
    seg_of_col = np.searchsorted(labp, labp, side="left")   # start idx per col
    pen_val = np.float16(128.0)

    ncls_max = max(
        len(np.unique(labp[c * ROWS:(c + 1) * ROWS])) for c in range(N_CORES))
    kp = KP if ncls_max <= NCLASS_MAX else 128
    assert ncls_max <= 126, ncls_max

    in_maps = []
    for c in range(N_CORES):
        r0, r1 = c * ROWS, (c + 1) * ROWS
        # distinct classes among this core's rows
        cls_ids = np.unique(labp[r0:r1])
        dim_of = {q: i for i, q in enumerate(cls_ids)}

        penl_a = np.zeros((kp, ROWS), dtype=np.float16)
        for i in range(ROWS):
            penl_a[dim_of[labp[r0 + i]], i] = pen_val
        penl_a[kp - 2, :] = np.float16(1.0)
        penl_a[kp - 1, :] = np.float16(1.0)

        penr_a = np.zeros((kp, B), dtype=np.float16)
        for q in cls_ids:
            qi = np.searchsorted(uniq, q)
            a, b = bounds[qi], bounds[qi + 1]
            penr_a[dim_of[q], a:b] = pen_val
        penr_a[kp - 2, :] = colhi
        penr_a[kp - 1, :] = collo

        in_maps.append({
            "et": et16,
            "eblk": np.ascontiguousarray(-et16[:, r0:r1]),
            "penl": penl_a,
            "penr": penr_a,
        })
    return in_maps, labp, rowterm, kp


def _postprocess(results, labp, rowterm):
    hp_raw = np.concatenate([r["out"][0][:MT].reshape(-1) for r in results])
    hn_raw = np.concatenate([r["out"][0][MT:].reshape(-1) for r in results])
    hp2 = hp_raw - np.float32(PEN) + rowterm
    hn2 = hn_raw + rowterm
    hp = np.sqrt(np.maximum(hp2, 0.0, dtype=np.float32))
    hn = np.sqrt(np.maximum(hn2, 0.0, dtype=np.float32))

    cnt_lab = np.bincount(labp, minlength=1)
    n_same = cnt_lab[labp]
    valid = (n_same > 1) & (n_same < B)
    per = np.where(valid, np.maximum(hp - hn + np.float32(MARGIN), 0.0), 0.0)
    cnt = np.float32(valid.sum())
    if cnt > 0:
        loss = np.float32(per.sum(dtype=np.float32) / max(cnt, np.float32(1.0)))
    else:
        loss = np.float32(0.0)
    return np.asarray(loss, dtype=np.float32)


def _run(in_maps, reps=1, kp=KP, **kw):
    nc = _get_nc(reps, kp)
    return run_bass_kernel_spmd(nc, in_maps, core_ids=list(range(N_CORES)), **kw)


def kernel(embeddings, labels):
    in_maps, labp, rowterm, kp = _prepare_inputs(embeddings, labels)
    res = _run(in_maps, kp=kp)
    return _postprocess(res.results, labp, rowterm)

